# revision 44
# baseline (speedup 1.0000x reference)
"""Trainium2 Bass kernel for nn_Decoding_43404939493634 (gnn_message_passing).

Reference computation:
    Z_a = node_embedding[actions_idx]            # [B, 64] gather
    s   = state_embedding @ W_4                  # [B, 1]
    Q   = relu(Z_a * s) @ W_5                    # [B, 1]

Algebraic restructuring: for a row with scalar s,
    relu(z * s) @ W5 = s * (relu(z) @ W5)        if s > 0
                     = s * (min(z, 0) @ W5)      if s <= 0
so with a per-node pair  A[v] = (relu(node_v) @ W5, min(node_v, 0) @ W5)
(computed on device), the per-batch-row work collapses to
    Q[b] = s[b] * (s[b] > 0 ? A[idx[b]].0 : A[idx[b]].1)

All 64-wide dot products run on the Tensor engine: the host stages
state/node shards TRANSPOSED in a "slot" layout [128, cols] where
column n of chunk c holds two rows' embeddings (partitions 0-63 /
64-127).  A bf16 matmul with a tiny block-diagonal stationary (16
patterns x 4 PE column groups) lands each chunk's 1024 dot products on
its own pair of PSUM partitions, filling [128, 512] PSUM banks.
f32->bf16 conversion happens inline in the DMA (SWDGE cast), so no
engine pass touches the full stream except PE.  min(z,0)@W5 is derived
as z@W5 - relu(z)@W5 (one ScalarE relu pass + 2 matmuls).

Two SPMD launches on 8 cores (both DMA-bound; measured 89-98 us total
(~36 + ~53 at best, +-7 us device noise) vs ~55 us pure-HBM roofline
and 211 us for the previous DVE/dma_gather implementation; each launch
carries ~12 us of fixed NEFF preamble/barrier cost):
  launch 1 (nodes): streams only the REFERENCED node rows (~21.6k/core,
      5.8 MB/core) -> A0/A1 tables.
  launch 2 (state+combine): host stages per-row A-pairs t2 =
      tbl[actions_idx] (data movement); device streams state
      (12.8 MB/core) -> s, then q = s * ((s>0)*(A0-A1) + A1), per
      PSUM-bank-group pipelined.

Host work is data movement only (pad/transpose/permute/take/unique);
every arithmetic op runs on device.
"""

import sys

for _p in ("/opt/trn_rl_repo",):
    if _p not in sys.path:
        sys.path.insert(0, _p)

import numpy as np

import concourse.bacc as bacc
import concourse.mybir as mybir
import concourse.tile as tile

F32 = mybir.dt.float32
BF16 = mybir.dt.bfloat16
ALU = mybir.AluOpType
RELU = mybir.ActivationFunctionType.Relu
P = 128

N_NODES = 200000
BATCH = 400000
EMB = 64
NCORES = 8

BATCH_PC = BATCH // NCORES           # 50000 rows/core
NODE_PC = N_NODES // NCORES          # 25000 nodes/core

FD = 512                             # matmul moving free dim / psum bank cols
CHUNK_ROWS = 2 * FD                  # rows ("slots") covered per matmul

S_CHUNKS = -(-BATCH_PC // CHUNK_ROWS)   # 49
S_SLOTS = S_CHUNKS * CHUNK_ROWS         # 50176
S_COLS = S_CHUNKS * FD                  # 25088

N_CHUNKS = 22                        # compacted: only referenced nodes staged
N_SLOTS = N_CHUNKS * CHUNK_ROWS         # 22528
N_COLS = N_CHUNKS * FD                  # 11264

DMA_COLS = 4096                      # 2 MiB (f32) per streaming DMA
HEAD_COLS = 1024                     # first tiles via HWDGE f32 + DVE cast


def _nc(num_devices):
    return bacc.Bacc(
        "TRN2", target_bir_lowering=False, debug=False, num_devices=num_devices
    )


def _dma_tiles(total_cols, first=0):
    """Streaming schedule: small first tiles (short SWDGE descriptor-gen +
    completion latency to the first matmul), then 2 MiB tiles.  `first`
    hoists the tail columns [first, total_cols) to the front so the PSUM
    group they complete is evacuated early, off the combine tail."""
    sched = []
    if first:
        sched.append((first, total_cols - first))
        total_cols = first
    c0 = 0
    for w in (FD, HEAD_COLS):
        if c0 < total_cols:
            cw = min(w, total_cols - c0)
            sched.append((c0, cw))
            c0 += cw
    # reserve small tail tiles so the last chunks' data lands (and their
    # matmuls + evacuation finish) right behind the stream, not 3-4us after
    tail = []
    for w in (FD, HEAD_COLS):
        if total_cols - c0 > w:
            tail.append(w)
    tail_cols = sum(tail)
    while c0 < total_cols - tail_cols:
        cw = min(DMA_COLS, total_cols - tail_cols - c0)
        sched.append((c0, cw))
        c0 += cw
    for w in reversed(tail):
        sched.append((c0, w))
        c0 += w
    return sched


def _stream_tile(nc, wpool, dram, c0, cw, tag):
    """Cast-DMA [128, cw] f32 from `dram` into a bf16 SBUF tile (the f32->
    bf16 conversion happens inline in the SWDGE DMA; no engine pass)."""
    tb = wpool.tile([P, cw], BF16, tag=f"{tag}b")
    nc.gpsimd.dma_start(out=tb[:], in_=dram[:, c0:c0 + cw])
    return tb


def build_nodes(n_chunks=N_CHUNKS, num_devices=NCORES):
    """Launch 1: A0 = relu(node)@W5, A1 = node@W5 - A0, slot layout."""
    n_cols = n_chunks * FD
    nc = _nc(num_devices)
    ndT = nc.declare_dram_parameter("ndT", [P, n_cols], F32, isOutput=False)
    patw5 = nc.declare_dram_parameter("patw5", [P, FD], BF16, isOutput=False)
    a0_out = nc.declare_dram_parameter("a0_out", [P, FD], F32, isOutput=True)
    a1_out = nc.declare_dram_parameter("a1_out", [P, FD], F32, isOutput=True)

    n_groups = -(-n_chunks // 16)        # psum banks each for a0 / s5

    with tile.TileContext(nc) as tc:
        with (
            tc.tile_pool(name="const", bufs=1) as cpool,
            tc.tile_pool(name="work", bufs=3) as wpool,
            tc.tile_pool(name="psum", bufs=1, space="PSUM") as ppool,
        ):
            p5 = cpool.tile([P, FD], BF16, tag="p5")
            nc.sync.dma_start(out=p5[:], in_=patw5[:])

            ps_a0 = [ppool.tile([P, FD], F32, tag=f"ps_a0{g}", name=f"ps_a0{g}")
                     for g in range(n_groups)]
            ps_s5 = [ppool.tile([P, FD], F32, tag=f"ps_s5{g}", name=f"ps_s5{g}")
                     for g in range(n_groups)]

            # PE warmup: ~3.4us of dummy matmuls during the head window
            # (while patterns/first tile are still in flight) so the HAM
            # clock-gate releases (1.2 -> 2.4 GHz) before the real matmuls;
            # the MM phase otherwise runs cold and overshoots the stream.
            warm = cpool.tile([P, FD], BF16, tag="warm")
            nc.vector.memset(warm[:], 0.0)
            ps_w = ppool.tile([P, FD], F32, tag="ps_w")
            for _ in range(7):
                nc.tensor.matmul(
                    ps_w[0:32, :], warm[:, 0:32], warm[:],
                    start=True, stop=True, skip_group_check=True,
                    tile_position=(0, 0),
                )

            for c0, cw in _dma_tiles(n_cols):
                ndb = _stream_tile(nc, wpool, ndT, c0, cw, "nd")
                rl = wpool.tile([P, cw], BF16, tag="rl")
                nc.scalar.activation(out=rl[:], in_=ndb[:], func=RELU)
                base = c0 // FD
                for k in range(cw // FD):
                    c = base + k
                    g, j = divmod(c, 16)
                    st_flags = dict(
                        start=(j == 0),
                        stop=(j == 15) or (c == n_chunks - 1),
                        skip_group_check=True,
                        tile_position=(0, 32 * g),
                    )
                    nc.tensor.matmul(
                        ps_a0[g][32 * g:32 * g + 32, :],
                        p5[:, 32 * j:32 * j + 32],
                        rl[:, k * FD:(k + 1) * FD],
                        **st_flags,
                    )
                    nc.tensor.matmul(
                        ps_s5[g][32 * g:32 * g + 32, :],
                        p5[:, 32 * j:32 * j + 32],
                        ndb[:, k * FD:(k + 1) * FD],
                        **st_flags,
                    )

            a0_sb = cpool.tile([P, FD], F32, tag="a0_sb")
            a1_sb = cpool.tile([P, FD], F32, tag="a1_sb")
            for g in range(n_groups):
                sl = slice(32 * g, 32 * g + 32)
                nc.vector.tensor_copy(out=a0_sb[sl, :], in_=ps_a0[g][sl, :])
                nc.vector.tensor_tensor(
                    out=a1_sb[sl, :], in0=ps_s5[g][sl, :], in1=a0_sb[sl, :],
                    op=ALU.subtract,
                )
                nc.sync.dma_start(out=a0_out[sl, :], in_=a0_sb[sl, :])
                nc.sync.dma_start(out=a1_out[sl, :], in_=a1_sb[sl, :])
    nc.compile()
    return nc


def build_state_combine(num_devices=NCORES):
    """Launch 2: s = state@W4 (slot layout), q = s*((s>0)*(g0-g1)+g1)."""
    nc = _nc(num_devices)
    stT = nc.declare_dram_parameter("stT", [P, S_COLS], F32, isOutput=False)
    patw4 = nc.declare_dram_parameter("patw4", [P, FD], BF16, isOutput=False)
    t2 = nc.declare_dram_parameter("t2", [P, FD, 2], F32, isOutput=False)
    q = nc.declare_dram_parameter("q", [P, FD], F32, isOutput=True)

    s_groups = -(-S_CHUNKS // 16)        # 4 psum banks

    with tile.TileContext(nc) as tc:
        with (
            tc.tile_pool(name="const", bufs=1) as cpool,
            tc.tile_pool(name="work", bufs=3) as wpool,
            tc.tile_pool(name="psum", bufs=1, space="PSUM") as ppool,
        ):
            p4 = cpool.tile([P, FD], BF16, tag="p4")
            nc.sync.dma_start(out=p4[:], in_=patw4[:])
            t2t = cpool.tile([P, FD, 2], F32, tag="t2t")
            nc.sync.dma_start(out=t2t[:], in_=t2[:])

            ps_s = [ppool.tile([P, FD], F32, tag=f"ps_s{g}", name=f"ps_s{g}")
                    for g in range(s_groups)]

            # the lone group-3 chunk (c=48, cols [24576, 25088)) streams
            # FIRST so its combine happens early, not on the tail
            for c0, cw in _dma_tiles(S_COLS, first=(S_CHUNKS - 1) * FD):
                stb = _stream_tile(nc, wpool, stT, c0, cw, "st")
                base = c0 // FD
                for k in range(cw // FD):
                    c = base + k
                    g, j = divmod(c, 16)
                    nc.tensor.matmul(
                        ps_s[g][32 * g:32 * g + 32, :],
                        p4[:, 32 * j:32 * j + 32],
                        stb[:, k * FD:(k + 1) * FD],
                        start=(j == 0),
                        stop=(j == 15) or (c == S_CHUNKS - 1),
                        skip_group_check=True,
                        tile_position=(0, 32 * g),
                    )

            # combine, pipelined per psum-bank group (32-partition slices);
            # full-height tiles so all DVE operands share a base partition
            sv = cpool.tile([P, FD], F32, tag="sv")
            d01 = cpool.tile([P, FD], F32, tag="d01")
            posm = cpool.tile([P, FD], F32, tag="posm")
            sel = cpool.tile([P, FD], F32, tag="sel")
            qt = cpool.tile([P, FD], F32, tag="qt")
            # d01 depends only on t2 -> compute during the stream, off the tail
            nc.vector.tensor_tensor(
                out=d01[:], in0=t2t[:, :, 0], in1=t2t[:, :, 1], op=ALU.subtract
            )
            # groups in stream-completion order (g3's chunk streams first);
            # DVE/Sync queues are FIFO, so emission order must match
            for g in [s_groups - 1] + list(range(s_groups - 1)):
                sl = slice(32 * g, 32 * g + 32)
                nc.vector.tensor_copy(out=sv[sl, :], in_=ps_s[g][sl, :])
                nc.vector.scalar_tensor_tensor(
                    out=posm[sl, :], in0=sv[sl, :], scalar=0.0, in1=d01[sl, :],
                    op0=ALU.is_gt, op1=ALU.mult,
                )
                nc.vector.tensor_tensor(
                    out=sel[sl, :], in0=posm[sl, :], in1=t2t[sl, :, 1],
                    op=ALU.add,
                )
                nc.vector.tensor_tensor(
                    out=qt[sl, :], in0=sv[sl, :], in1=sel[sl, :], op=ALU.mult
                )
                nc.sync.dma_start(out=q[sl, :], in_=qt[sl, :])
    nc.compile()
    return nc


# ---------------------------------------------------------------------------
# host-side staging (data movement only) + execution

_CACHE = {}
LAST_RUNS = []  # BassKernelResults of each launch in the last kernel() call


def _runner(key, build_fn):
    if key not in _CACHE:
        _CACHE[key] = build_fn()
    return _CACHE[key]


def _run_spmd(nc, in_maps):
    from concourse.bass_utils import run_bass_kernel_spmd

    r = run_bass_kernel_spmd(nc, in_maps, core_ids=list(range(NCORES)))
    LAST_RUNS.append(r)
    return r.results


def _slotT(rows, n_slots, nch):
    """[n, 64] -> transposed slot layout [128, n_slots//2]: column of chunk c,
    col n holds rows (1024c+2n) on partitions 0-63 and (1024c+2n+1) on
    64-127."""
    n = rows.shape[0]
    buf = np.zeros((n_slots, EMB), np.float32)
    buf[:n] = rows
    arr = buf.reshape(nch, FD, 2, EMB)           # [c, n, h, e]
    return np.ascontiguousarray(
        arr.transpose(2, 3, 0, 1).reshape(P, nch * FD)
    )


def _pidx(n_chunks):
    """Partition index of (chunk c, half h) in the psum/slot output layout."""
    c = np.arange(n_chunks)[:, None]
    h = np.arange(2)[None, :]
    return (32 * (c // 16) + 2 * (c % 16) + h)   # [n_chunks, 2]


def _unslot(mat, n_chunks):
    """[128, 512] device output -> flat [n_chunks*1024] slot-ordered values."""
    pi = _pidx(n_chunks).reshape(-1)             # [2*n_chunks]
    v = mat[pi, :].reshape(n_chunks, 2, FD)      # [c, h, n]
    return np.ascontiguousarray(v.transpose(0, 2, 1)).reshape(-1)


def _slot_pairs(pairs, n_chunks):
    """[n_slots, 2] per-slot values -> [128, 512, 2] device layout."""
    pi = _pidx(n_chunks).reshape(-1)
    arr = pairs.reshape(n_chunks, FD, 2, 2)      # [c, n, h, v]
    out = np.zeros((P, FD, 2), np.float32)
    out[pi] = arr.transpose(0, 2, 1, 3).reshape(2 * n_chunks, FD, 2)
    return out


def _patterns(w):
    """16 block-diagonal stationaries packed as [128, 512] bf16: pattern j in
    cols [32j, 32j+32) with w at (rows 0-63, col 2j), (rows 64-127, col
    2j+1)."""
    pat = np.zeros((P, FD), np.float32)
    for j in range(16):
        pat[:EMB, 32 * j + 2 * j] = w
        pat[EMB:, 32 * j + 2 * j + 1] = w
    return pat


def kernel(actions_idx, node_embedding, state_embedding, W_4, W_5):
    LAST_RUNS.clear()
    actions_idx = np.asarray(actions_idx)
    node_embedding = np.ascontiguousarray(np.asarray(node_embedding, dtype=np.float32))
    state_embedding = np.ascontiguousarray(np.asarray(state_embedding, dtype=np.float32))
    w4 = np.asarray(W_4, dtype=np.float32).reshape(EMB)
    w5 = np.asarray(W_5, dtype=np.float32).reshape(EMB)
    bf16 = mybir.dt.np(BF16)
    patw4 = _patterns(w4).astype(bf16)
    patw5 = _patterns(w5).astype(bf16)

    # ---- launch 1: node stream -> A tables (only referenced nodes staged)
    uniq, inv = np.unique(actions_idx, return_inverse=True)
    u_pc = -(-len(uniq) // NCORES)               # referenced nodes per core
    n_chunks = max(1, -(-u_pc // CHUNK_ROWS))    # 22 for the target workload
    n_slots = n_chunks * CHUNK_ROWS
    nc1 = _runner(("nodes", n_chunks), lambda: build_nodes(n_chunks))
    in1 = []
    for c in range(NCORES):
        rows = node_embedding[uniq[c * u_pc:(c + 1) * u_pc]]
        in1.append({"ndT": _slotT(rows, n_slots, n_chunks), "patw5": patw5})
    res1 = _run_spmd(nc1, in1)

    tblu = np.empty((NCORES * u_pc, 2), np.float32)
    for c in range(NCORES):
        sl = slice(c * u_pc, (c + 1) * u_pc)
        tblu[sl, 0] = _unslot(res1[c]["a0_out"], n_chunks)[:u_pc]
        tblu[sl, 1] = _unslot(res1[c]["a1_out"], n_chunks)[:u_pc]

    # ---- launch 2: state stream + combine
    nc2 = _runner("state", build_state_combine)
    in2 = []
    for c in range(NCORES):
        cinv = inv[c * BATCH_PC:(c + 1) * BATCH_PC]
        pairs = np.zeros((S_SLOTS, 2), np.float32)
        pairs[:BATCH_PC] = tblu[cinv]
        in2.append({
            "stT": _slotT(state_embedding[c * BATCH_PC:(c + 1) * BATCH_PC],
                          S_SLOTS, S_CHUNKS),
            "patw4": patw4,
            "t2": _slot_pairs(pairs, S_CHUNKS),
        })
    res2 = _run_spmd(nc2, in2)

    out = np.empty(BATCH, np.float32)
    for c in range(NCORES):
        out[c * BATCH_PC:(c + 1) * BATCH_PC] = \
            _unslot(res2[c]["q"], S_CHUNKS)[:BATCH_PC]
    return out.reshape(BATCH, 1)


# revision 45
# speedup vs baseline: 1.1310x; 1.1310x over previous
"""Trainium2 Bass kernel for nn_Decoding_43404939493634 (gnn_message_passing).

Reference computation:
    Z_a = node_embedding[actions_idx]            # [B, 64] gather
    s   = state_embedding @ W_4                  # [B, 1]
    Q   = relu(Z_a * s) @ W_5                    # [B, 1]

Algebraic restructuring: for a row with scalar s,
    relu(z * s) @ W5 = s * (relu(z) @ W5)        if s > 0
                     = s * (min(z, 0) @ W5)      if s <= 0
so with a per-node pair  A[v] = (relu(node_v) @ W5, min(node_v, 0) @ W5)
(computed on device), the per-batch-row work collapses to
    Q[b] = s[b] * (s[b] > 0 ? A[idx[b]].0 : A[idx[b]].1)

All 64-wide dot products run on the Tensor engine: the host stages
state/node shards TRANSPOSED in a "slot" layout [128, cols] where
column n of chunk c holds two rows' embeddings (partitions 0-63 /
64-127).  A bf16 matmul with a tiny block-diagonal stationary (16
patterns x 4 PE column groups) lands each chunk's 1024 dot products on
its own pair of PSUM partitions, filling [128, 512] PSUM banks.
f32->bf16 conversion happens inline in the DMA (SWDGE cast), so no
engine pass touches the full stream except PE.  min(z,0)@W5 is derived
as z@W5 - relu(z)@W5 (one ScalarE relu pass + 2 matmuls).

Two SPMD launches on 8 cores (both DMA-bound; measured 89-98 us total
(~36 + ~53 at best, +-7 us device noise) vs ~55 us pure-HBM roofline
and 211 us for the previous DVE/dma_gather implementation; each launch
carries ~12 us of fixed NEFF preamble/barrier cost):
  launch 1 (nodes): streams only the REFERENCED node rows (~21.6k/core,
      5.8 MB/core) -> A0/A1 tables.
  launch 2 (state+combine): host stages per-row A-pairs t2 =
      tbl[actions_idx] (data movement); device streams state
      (12.8 MB/core) -> s, then q = s * ((s>0)*(A0-A1) + A1), per
      PSUM-bank-group pipelined.

Host work is data movement only (pad/transpose/permute/take/unique);
every arithmetic op runs on device.
"""

import sys

for _p in ("/opt/trn_rl_repo",):
    if _p not in sys.path:
        sys.path.insert(0, _p)

import numpy as np

import concourse.bacc as bacc
import concourse.mybir as mybir
import concourse.tile as tile

F32 = mybir.dt.float32
BF16 = mybir.dt.bfloat16
ALU = mybir.AluOpType
RELU = mybir.ActivationFunctionType.Relu
P = 128

N_NODES = 200000
BATCH = 400000
EMB = 64
NCORES = 8

BATCH_PC = BATCH // NCORES           # 50000 rows/core
NODE_PC = N_NODES // NCORES          # 25000 nodes/core

FD = 512                             # matmul moving free dim / psum bank cols
CHUNK_ROWS = 2 * FD                  # rows ("slots") covered per matmul

S_CHUNKS = -(-BATCH_PC // CHUNK_ROWS)   # 49
S_SLOTS = S_CHUNKS * CHUNK_ROWS         # 50176
S_COLS = S_CHUNKS * FD                  # 25088

N_CHUNKS = 22                        # compacted: only referenced nodes staged
N_SLOTS = N_CHUNKS * CHUNK_ROWS         # 22528
N_COLS = N_CHUNKS * FD                  # 11264

DMA_COLS = 4096                      # 2 MiB (f32) per streaming DMA
HEAD_COLS = 1024                     # first tiles via HWDGE f32 + DVE cast


def _nc(num_devices):
    return bacc.Bacc(
        "TRN2", target_bir_lowering=False, debug=False, num_devices=num_devices
    )


def _dma_tiles(total_cols, first=0):
    """Streaming schedule: small first tiles (short SWDGE descriptor-gen +
    completion latency to the first matmul), then 2 MiB tiles.  `first`
    hoists the tail columns [first, total_cols) to the front so the PSUM
    group they complete is evacuated early, off the combine tail."""
    sched = []
    if first:
        sched.append((first, total_cols - first))
        total_cols = first
    c0 = 0
    for w in (FD, HEAD_COLS):
        if c0 < total_cols:
            cw = min(w, total_cols - c0)
            sched.append((c0, cw))
            c0 += cw
    while c0 < total_cols:
        cw = min(DMA_COLS, total_cols - c0)
        sched.append((c0, cw))
        c0 += cw
    return sched


def _stream_tile(nc, wpool, dram, c0, cw, tag):
    """Cast-DMA [128, cw] f32 from `dram` into a bf16 SBUF tile (the f32->
    bf16 conversion happens inline in the SWDGE DMA; no engine pass)."""
    tb = wpool.tile([P, cw], BF16, tag=f"{tag}b")
    nc.gpsimd.dma_start(out=tb[:], in_=dram[:, c0:c0 + cw])
    return tb


def build_nodes(n_chunks=N_CHUNKS, num_devices=NCORES):
    """Launch 1: A0 = relu(node)@W5, A1 = node@W5 - A0, slot layout."""
    n_cols = n_chunks * FD
    nc = _nc(num_devices)
    ndT = nc.declare_dram_parameter("ndT", [P, n_cols], F32, isOutput=False)
    patw5 = nc.declare_dram_parameter("patw5", [P, FD], BF16, isOutput=False)
    a0_out = nc.declare_dram_parameter("a0_out", [P, FD], F32, isOutput=True)
    a1_out = nc.declare_dram_parameter("a1_out", [P, FD], F32, isOutput=True)

    n_groups = -(-n_chunks // 16)        # psum banks each for a0 / s5

    with tile.TileContext(nc) as tc:
        with (
            tc.tile_pool(name="const", bufs=1) as cpool,
            tc.tile_pool(name="work", bufs=3) as wpool,
            tc.tile_pool(name="psum", bufs=1, space="PSUM") as ppool,
        ):
            p5 = cpool.tile([P, FD], BF16, tag="p5")
            nc.sync.dma_start(out=p5[:], in_=patw5[:])

            ps_a0 = [ppool.tile([P, FD], F32, tag=f"ps_a0{g}", name=f"ps_a0{g}")
                     for g in range(n_groups)]
            ps_s5 = [ppool.tile([P, FD], F32, tag=f"ps_s5{g}", name=f"ps_s5{g}")
                     for g in range(n_groups)]

            # PE warmup: ~3.4us of dummy matmuls during the head window
            # (while patterns/first tile are still in flight) so the HAM
            # clock-gate releases (1.2 -> 2.4 GHz) before the real matmuls;
            # the MM phase otherwise runs cold and overshoots the stream.
            warm = cpool.tile([P, FD], BF16, tag="warm")
            nc.vector.memset(warm[:], 0.0)
            ps_w = ppool.tile([P, FD], F32, tag="ps_w")
            for _ in range(7):
                nc.tensor.matmul(
                    ps_w[0:32, :], warm[:, 0:32], warm[:],
                    start=True, stop=True, skip_group_check=True,
                    tile_position=(0, 0),
                )

            for c0, cw in _dma_tiles(n_cols):
                ndb = _stream_tile(nc, wpool, ndT, c0, cw, "nd")
                rl = wpool.tile([P, cw], BF16, tag="rl")
                nc.scalar.activation(out=rl[:], in_=ndb[:], func=RELU)
                base = c0 // FD
                for k in range(cw // FD):
                    c = base + k
                    g, j = divmod(c, 16)
                    st_flags = dict(
                        start=(j == 0),
                        stop=(j == 15) or (c == n_chunks - 1),
                        skip_group_check=True,
                        tile_position=(0, 32 * g),
                    )
                    nc.tensor.matmul(
                        ps_a0[g][32 * g:32 * g + 32, :],
                        p5[:, 32 * j:32 * j + 32],
                        rl[:, k * FD:(k + 1) * FD],
                        **st_flags,
                    )
                    nc.tensor.matmul(
                        ps_s5[g][32 * g:32 * g + 32, :],
                        p5[:, 32 * j:32 * j + 32],
                        ndb[:, k * FD:(k + 1) * FD],
                        **st_flags,
                    )

            a0_sb = cpool.tile([P, FD], F32, tag="a0_sb")
            a1_sb = cpool.tile([P, FD], F32, tag="a1_sb")
            for g in range(n_groups):
                sl = slice(32 * g, 32 * g + 32)
                nc.vector.tensor_copy(out=a0_sb[sl, :], in_=ps_a0[g][sl, :])
                nc.vector.tensor_tensor(
                    out=a1_sb[sl, :], in0=ps_s5[g][sl, :], in1=a0_sb[sl, :],
                    op=ALU.subtract,
                )
                nc.sync.dma_start(out=a0_out[sl, :], in_=a0_sb[sl, :])
                nc.sync.dma_start(out=a1_out[sl, :], in_=a1_sb[sl, :])
    nc.compile()
    return nc


def build_state_combine(num_devices=NCORES):
    """Launch 2: s = state@W4 (slot layout), q = s*((s>0)*(g0-g1)+g1)."""
    nc = _nc(num_devices)
    stT = nc.declare_dram_parameter("stT", [P, S_COLS], F32, isOutput=False)
    patw4 = nc.declare_dram_parameter("patw4", [P, FD], BF16, isOutput=False)
    t2 = nc.declare_dram_parameter("t2", [P, FD, 2], F32, isOutput=False)
    q = nc.declare_dram_parameter("q", [P, FD], F32, isOutput=True)

    s_groups = -(-S_CHUNKS // 16)        # 4 psum banks

    with tile.TileContext(nc) as tc:
        with (
            tc.tile_pool(name="const", bufs=1) as cpool,
            tc.tile_pool(name="work", bufs=3) as wpool,
            tc.tile_pool(name="psum", bufs=1, space="PSUM") as ppool,
        ):
            p4 = cpool.tile([P, FD], BF16, tag="p4")
            nc.sync.dma_start(out=p4[:], in_=patw4[:])
            t2t = cpool.tile([P, FD, 2], F32, tag="t2t")
            nc.sync.dma_start(out=t2t[:], in_=t2[:])

            ps_s = [ppool.tile([P, FD], F32, tag=f"ps_s{g}", name=f"ps_s{g}")
                    for g in range(s_groups)]

            # the lone group-3 chunk (c=48, cols [24576, 25088)) streams
            # FIRST so its combine happens early, not on the tail
            for c0, cw in _dma_tiles(S_COLS, first=(S_CHUNKS - 1) * FD):
                stb = _stream_tile(nc, wpool, stT, c0, cw, "st")
                base = c0 // FD
                for k in range(cw // FD):
                    c = base + k
                    g, j = divmod(c, 16)
                    nc.tensor.matmul(
                        ps_s[g][32 * g:32 * g + 32, :],
                        p4[:, 32 * j:32 * j + 32],
                        stb[:, k * FD:(k + 1) * FD],
                        start=(j == 0),
                        stop=(j == 15) or (c == S_CHUNKS - 1),
                        skip_group_check=True,
                        tile_position=(0, 32 * g),
                    )

            # combine, pipelined per psum-bank group (32-partition slices);
            # full-height tiles so all DVE operands share a base partition
            sv = cpool.tile([P, FD], F32, tag="sv")
            d01 = cpool.tile([P, FD], F32, tag="d01")
            posm = cpool.tile([P, FD], F32, tag="posm")
            sel = cpool.tile([P, FD], F32, tag="sel")
            qt = cpool.tile([P, FD], F32, tag="qt")
            # d01 depends only on t2 -> compute during the stream, off the tail
            nc.vector.tensor_tensor(
                out=d01[:], in0=t2t[:, :, 0], in1=t2t[:, :, 1], op=ALU.subtract
            )
            # groups in stream-completion order (g3's chunk streams first);
            # DVE/Sync queues are FIFO, so emission order must match
            for g in [s_groups - 1] + list(range(s_groups - 1)):
                sl = slice(32 * g, 32 * g + 32)
                nc.vector.tensor_copy(out=sv[sl, :], in_=ps_s[g][sl, :])
                nc.vector.scalar_tensor_tensor(
                    out=posm[sl, :], in0=sv[sl, :], scalar=0.0, in1=d01[sl, :],
                    op0=ALU.is_gt, op1=ALU.mult,
                )
                nc.vector.tensor_tensor(
                    out=sel[sl, :], in0=posm[sl, :], in1=t2t[sl, :, 1],
                    op=ALU.add,
                )
                nc.vector.tensor_tensor(
                    out=qt[sl, :], in0=sv[sl, :], in1=sel[sl, :], op=ALU.mult
                )
                nc.sync.dma_start(out=q[sl, :], in_=qt[sl, :])
    nc.compile()
    return nc


# ---------------------------------------------------------------------------
# host-side staging (data movement only) + execution

_CACHE = {}
LAST_RUNS = []  # BassKernelResults of each launch in the last kernel() call


def _runner(key, build_fn):
    if key not in _CACHE:
        _CACHE[key] = build_fn()
    return _CACHE[key]


def _run_spmd(nc, in_maps):
    from concourse.bass_utils import run_bass_kernel_spmd

    r = run_bass_kernel_spmd(nc, in_maps, core_ids=list(range(NCORES)))
    LAST_RUNS.append(r)
    return r.results


def _slotT(rows, n_slots, nch):
    """[n, 64] -> transposed slot layout [128, n_slots//2]: column of chunk c,
    col n holds rows (1024c+2n) on partitions 0-63 and (1024c+2n+1) on
    64-127."""
    n = rows.shape[0]
    buf = np.zeros((n_slots, EMB), np.float32)
    buf[:n] = rows
    arr = buf.reshape(nch, FD, 2, EMB)           # [c, n, h, e]
    return np.ascontiguousarray(
        arr.transpose(2, 3, 0, 1).reshape(P, nch * FD)
    )


def _pidx(n_chunks):
    """Partition index of (chunk c, half h) in the psum/slot output layout."""
    c = np.arange(n_chunks)[:, None]
    h = np.arange(2)[None, :]
    return (32 * (c // 16) + 2 * (c % 16) + h)   # [n_chunks, 2]


def _unslot(mat, n_chunks):
    """[128, 512] device output -> flat [n_chunks*1024] slot-ordered values."""
    pi = _pidx(n_chunks).reshape(-1)             # [2*n_chunks]
    v = mat[pi, :].reshape(n_chunks, 2, FD)      # [c, h, n]
    return np.ascontiguousarray(v.transpose(0, 2, 1)).reshape(-1)


def _slot_pairs(pairs, n_chunks):
    """[n_slots, 2] per-slot values -> [128, 512, 2] device layout."""
    pi = _pidx(n_chunks).reshape(-1)
    arr = pairs.reshape(n_chunks, FD, 2, 2)      # [c, n, h, v]
    out = np.zeros((P, FD, 2), np.float32)
    out[pi] = arr.transpose(0, 2, 1, 3).reshape(2 * n_chunks, FD, 2)
    return out


def _patterns(w):
    """16 block-diagonal stationaries packed as [128, 512] bf16: pattern j in
    cols [32j, 32j+32) with w at (rows 0-63, col 2j), (rows 64-127, col
    2j+1)."""
    pat = np.zeros((P, FD), np.float32)
    for j in range(16):
        pat[:EMB, 32 * j + 2 * j] = w
        pat[EMB:, 32 * j + 2 * j + 1] = w
    return pat


def kernel(actions_idx, node_embedding, state_embedding, W_4, W_5):
    LAST_RUNS.clear()
    actions_idx = np.asarray(actions_idx)
    node_embedding = np.ascontiguousarray(np.asarray(node_embedding, dtype=np.float32))
    state_embedding = np.ascontiguousarray(np.asarray(state_embedding, dtype=np.float32))
    w4 = np.asarray(W_4, dtype=np.float32).reshape(EMB)
    w5 = np.asarray(W_5, dtype=np.float32).reshape(EMB)
    bf16 = mybir.dt.np(BF16)
    patw4 = _patterns(w4).astype(bf16)
    patw5 = _patterns(w5).astype(bf16)

    # ---- launch 1: node stream -> A tables (only referenced nodes staged)
    uniq, inv = np.unique(actions_idx, return_inverse=True)
    u_pc = -(-len(uniq) // NCORES)               # referenced nodes per core
    n_chunks = max(1, -(-u_pc // CHUNK_ROWS))    # 22 for the target workload
    n_slots = n_chunks * CHUNK_ROWS
    nc1 = _runner(("nodes", n_chunks), lambda: build_nodes(n_chunks))
    in1 = []
    for c in range(NCORES):
        rows = node_embedding[uniq[c * u_pc:(c + 1) * u_pc]]
        in1.append({"ndT": _slotT(rows, n_slots, n_chunks), "patw5": patw5})
    res1 = _run_spmd(nc1, in1)

    tblu = np.empty((NCORES * u_pc, 2), np.float32)
    for c in range(NCORES):
        sl = slice(c * u_pc, (c + 1) * u_pc)
        tblu[sl, 0] = _unslot(res1[c]["a0_out"], n_chunks)[:u_pc]
        tblu[sl, 1] = _unslot(res1[c]["a1_out"], n_chunks)[:u_pc]

    # ---- launch 2: state stream + combine
    nc2 = _runner("state", build_state_combine)
    in2 = []
    for c in range(NCORES):
        cinv = inv[c * BATCH_PC:(c + 1) * BATCH_PC]
        pairs = np.zeros((S_SLOTS, 2), np.float32)
        pairs[:BATCH_PC] = tblu[cinv]
        in2.append({
            "stT": _slotT(state_embedding[c * BATCH_PC:(c + 1) * BATCH_PC],
                          S_SLOTS, S_CHUNKS),
            "patw4": patw4,
            "t2": _slot_pairs(pairs, S_CHUNKS),
        })
    res2 = _run_spmd(nc2, in2)

    out = np.empty(BATCH, np.float32)
    for c in range(NCORES):
        out[c * BATCH_PC:(c + 1) * BATCH_PC] = \
            _unslot(res2[c]["q"], S_CHUNKS)[:BATCH_PC]
    return out.reshape(BATCH, 1)
